# revision 8
# baseline (speedup 1.0000x reference)
"""Embedding lookup kernel for Trainium2 (8 NeuronCores, data-parallel).

out[b, s, :] = emb_table[road_map[data[b, s, 0]]], zeros where data == PAD_ID.

This runtime supports indirect DMA only in its one-offset-per-partition form
(out[p, :] = in.flat[idx[p]*coef + 0..nepi]), so the kernel issues, per core
(65536 ids), 512 call pairs on the GPSIMD/pool engine:
  A_j: cids[:, j] = road_map2[ids[:, j]]      (128 scalars, 4B descriptors)
  B_j: rows[:, jj*128:+128] = emb2[cids[:, j]] (128 rows, 512B descriptors)
with HWDGE stores of 4 MiB output groups overlapped via double buffering.

Host staging is data-independent: road_map entry PAD -> 4096 plus a zero
row appended to the table (pad ids produce zeros without masking), and the
id stream reshaped so partition p of group t owns output rows
t*8192 + p*64 .. +64, making every store contiguous per partition.
"""

from contextlib import ExitStack

import numpy as np

import concourse.bass as bass
import concourse.mybir as mybir
from concourse.bass_utils import run_bass_kernel_spmd

B, S, E = 128, 4096, 128
N_CORES = 8
B_SH = B // N_CORES              # 16 batches per core
N_IDS = B_SH * S                 # 65536 ids per core
ROUTEID_NUM = 100000
RM_LEN = ROUTEID_NUM + 2
PAD_ID = ROUTEID_NUM + 1
CLUSTER_NUM = 4096
ZERO_ROW = CLUSTER_NUM

NCALL = N_IDS // 128             # 512 call pairs per core
T = 8                            # store groups
JT = NCALL // T                  # 64 calls per group
NI = 128 * JT                    # 8192 rows per group

_NC_CACHE = {}


def _build_bass():
    nc = bass.Bass()
    i32, f32 = mybir.dt.int32, mybir.dt.float32
    ids_d = nc.dram_tensor("ids", [128, NCALL], i32, kind="ExternalInput")
    rm_d = nc.dram_tensor("rm2", [RM_LEN, 1], i32, kind="ExternalInput")
    emb_d = nc.dram_tensor("emb2", [CLUSTER_NUM + 1, E], f32, kind="ExternalInput")
    out_d = nc.dram_tensor("out", [N_IDS, E], f32, kind="ExternalOutput")
    out_v = out_d[:, :].rearrange("(t p g) e -> t p (g e)", t=T, p=128)

    with ExitStack() as ctx:
        sb = lambda n, s, d: ctx.enter_context(nc.sbuf_tensor(n, s, d))
        sem = lambda n: ctx.enter_context(nc.semaphore(n))
        ids_sb = sb("ids_sb", [128, NCALL], i32)
        cids_sb = sb("cids_sb", [128, NCALL], i32)
        rows = [sb("rows0", [128, JT * E], f32), sb("rows1", [128, JT * E], f32)]
        sIn, sA, sB, sC = sem("sIn"), sem("sA"), sem("sB"), sem("sC")

        # sync engine: input load + output stores
        nc.sync.dma_start(ids_sb[:, :], ids_d[:, :]).then_inc(sIn, 16)
        for t in range(T):
            nc.sync.wait_ge(sB, 16 * JT * (t + 1))
            nc.sync.dma_start(out_v[t], rows[t % 2][:, :]).then_inc(sC, 16)

        # pool engine: all A gathers first (no waits), then B gathers
        nc.gpsimd.wait_ge(sIn, 16)
        for j in range(NCALL):
            nc.gpsimd.indirect_dma_start(
                out=cids_sb[:, j : j + 1],
                out_offset=None,
                in_=rm_d[:, :],
                in_offset=bass.IndirectOffsetOnAxis(ap=ids_sb[:, j : j + 1], axis=0),
            ).then_inc(sA, 16)
        nc.gpsimd.wait_ge(sA, 16 * NCALL)
        for jb in range(NCALL):
            t, jj = jb // JT, jb % JT
            if jj == 0 and t >= 2:
                nc.gpsimd.wait_ge(sC, 16 * (t - 1))      # rows[t%2] free
            nc.gpsimd.indirect_dma_start(
                out=rows[t % 2][:, jj * E : (jj + 1) * E],
                out_offset=None,
                in_=emb_d[:, :],
                in_offset=bass.IndirectOffsetOnAxis(
                    ap=cids_sb[:, jb : jb + 1], axis=0
                ),
            ).then_inc(sB, 16)
    return nc


def _stage_inputs(data, road_map, emb_table):
    data = np.asarray(data).reshape(B, S)
    road_map = np.asarray(road_map, dtype=np.int32)
    emb_table = np.asarray(emb_table, dtype=np.float32)

    rm2 = road_map.copy()
    rm2[PAD_ID] = ZERO_ROW
    rm2 = rm2.reshape(RM_LEN, 1)
    emb2 = np.concatenate([emb_table, np.zeros((1, E), np.float32)], axis=0)

    in_maps = []
    for c in range(N_CORES):
        shard = data[c * B_SH : (c + 1) * B_SH].reshape(-1).astype(np.int32)
        # ids[p, t*JT + jj] = shard[t*NI + p*JT + jj]
        ids = np.ascontiguousarray(
            shard.reshape(T, 128, JT).transpose(1, 0, 2).reshape(128, NCALL)
        )
        in_maps.append({"ids": ids, "rm2": rm2, "emb2": emb2})
    return in_maps


def kernel(data, road_map, emb_table, trace=False, **run_kwargs):
    if "nc" not in _NC_CACHE:
        _NC_CACHE["nc"] = _build_bass()
    nc = _NC_CACHE["nc"]
    in_maps = _stage_inputs(data, road_map, emb_table)
    import time

    t0 = time.time()
    res = run_bass_kernel_spmd(
        nc, in_maps, core_ids=list(range(N_CORES)), trace=trace, **run_kwargs
    )
    _NC_CACHE["spmd_wall_ns"] = int((time.time() - t0) * 1e9)
    out = np.empty((B, S, E), np.float32)
    for c in range(N_CORES):
        out[c * B_SH : (c + 1) * B_SH] = res.results[c]["out"].reshape(B_SH, S, E)
    _NC_CACHE["last_result"] = res
    return out
